# revision 27
# baseline (speedup 1.0000x reference)
"""GATv2 2-layer GNN message-passing kernel for Trainium2, 8-core SPMD (v2).

Contract: kernel(**inputs) takes FULL unsharded inputs and returns the FULL
[50000, 128] float32 output.

Strategy (edge/data parallel, dst-range sharded, descriptor-minimized):
- Host: append self-loops, sort edges by dst; each core owns 6250 dst nodes
  = 49 blocks of 128. Per block, edges are split by src-half (int16 gather
  index limit) and padded to 128-edge groups with per-block group counts.
- All tables and per-edge compute are bf16 (fp32 PSUM accumulation);
  absmax-rel error ~5e-3 vs fp32 (gate 2e-2).
- Only ONE dma_gather stream per edge (xl[src], 256B descriptors) spread
  over 4 SWDGE queues with a 64KB descriptor ring. xr[dst] is NOT gathered:
  dst is block-local, so xr rows are reconstructed on the PE as
  zb = ST_g @ xrb  (ST = transposed one-hot of dst_rel built on-chip).
- Per-edge pipeline: z = za+zb (DVE), LeakyReLU (ACT), att-dot (DVE
  mult+reduce), w = exp (ACT, written into comb), u = w*za (DVE, into comb),
  then one PE matmul chain scatters S^T @ [u|w] into the block PSUM.
  out = relu(psum_u / (psum_w+eps) + bias).
- Layer 2 reuses the SAME gather indices: the AllGather writes xl2 in global
  node order ([50000,128] bf16), and xr2 stays core-local.
"""
import sys
sys.path.insert(0, '/opt/trn_rl_repo')
import numpy as np
from dataclasses import dataclass, field

import concourse.bass as bass
import concourse.bacc as bacc
import concourse.mybir as mybir
from concourse.tile import TileContext
from concourse.library_config import mlp
from concourse.bass_utils import run_bass_kernel_spmd

P = 128
H, C = 4, 32
D = H * C          # 128
SLOPE = 0.2
F32 = mybir.dt.float32
BF16 = mybir.dt.bfloat16
I16 = mybir.dt.int16
NPBF = mybir.dt.np(BF16)


@dataclass
class Plan:
    N: int
    NC: int
    NPC: int            # nodes per core (6250)
    NBLK: int           # blocks per core (49)
    SPLIT: int          # lo/hi table split (25000)
    G_lo: list = field(default_factory=list)   # per-block lo group count
    G_hi: list = field(default_factory=list)   # per-block hi group count

    @property
    def G(self):
        return [a + b for a, b in zip(self.G_lo, self.G_hi)]

    @property
    def GMAX(self):
        return max(self.G)


def wrap_idx(flat):
    """[n] int -> dma_gather SBUF idx layout [128, n//16]."""
    n = flat.shape[0]
    assert n % 16 == 0
    w = flat.reshape(n // 16, 16).T
    return np.tile(w, (8, 1)).astype(np.int16)


def preprocess(x, edge_index, NC=8):
    N = x.shape[0]
    NPC = N // NC
    NBLK = (NPC + P - 1) // P
    SPLIT = N // 2

    loop = np.arange(N, dtype=np.int64)
    src = np.concatenate([np.asarray(edge_index[0]), loop]).astype(np.int64)
    dst = np.concatenate([np.asarray(edge_index[1]), loop]).astype(np.int64)
    order = np.argsort(dst, kind='stable')
    src = src[order].astype(np.int32)
    dst = dst[order].astype(np.int32)
    core_bounds = np.searchsorted(dst, np.arange(NC + 1) * NPC)

    per_core = []
    lo_counts = np.zeros((NC, NBLK), np.int64)
    hi_counts = np.zeros((NC, NBLK), np.int64)
    for k in range(NC):
        a, b = core_bounds[k], core_bounds[k + 1]
        s_k = src[a:b]
        d_k = dst[a:b] - k * NPC
        blk = d_k // P
        is_lo = s_k < SPLIT
        lo_counts[k] = np.bincount(blk[is_lo], minlength=NBLK)
        hi_counts[k] = np.bincount(blk[~is_lo], minlength=NBLK)
        per_core.append((s_k, d_k, blk, is_lo))

    plan = Plan(N=N, NC=NC, NPC=NPC, NBLK=NBLK, SPLIT=SPLIT)
    plan.G_lo = [int(v) for v in
                 (lo_counts.max(axis=0) + P - 1) // P]
    plan.G_hi = [int(v) for v in
                 (hi_counts.max(axis=0) + P - 1) // P]
    G = plan.G

    datas = []
    for k in range(NC):
        s_k, d_k, blk, is_lo = per_core[k]
        idx_cols, drt_cols, drrow_cols = [], [], []
        for b in range(NBLK):
            in_b = blk == b
            dr_flat = np.full(G[b] * P, -1.0, np.float32)
            wrapped = []
            for side, gn, off in ((True, plan.G_lo[b], 0),
                                  (False, plan.G_hi[b], plan.G_lo[b] * P)):
                flat = np.zeros(gn * P, np.int16)
                sel = in_b & (is_lo == side)
                ss = s_k[sel]
                n = ss.shape[0]
                assert n <= gn * P, (k, b, side, n, gn)
                flat[:n] = ss if side else ss - SPLIT
                dr_flat[off:off + n] = d_k[sel] - b * P
                if gn:
                    wrapped.append(wrap_idx(flat))
            idx_cols.append(np.concatenate(wrapped, axis=1))
            drt_cols.append(dr_flat.reshape(G[b], P).T)   # [128, G]
            drrow_cols.append(dr_flat.reshape(1, G[b] * P))
        datas.append(dict(
            idx_cat=np.ascontiguousarray(np.concatenate(idx_cols, axis=1)),
            drt_cat=np.ascontiguousarray(
                np.concatenate(drt_cols, axis=1)).astype(NPBF),
            drrow_cat=np.ascontiguousarray(
                np.concatenate(drrow_cols, axis=1)).astype(NPBF),
        ))
    return plan, datas


def build_kernel(plan, repeat=1, skip_collective=False, queues=1,
                 scratch=16384, gch=8):
    pl = plan
    NBLK, NPC, GMAX = pl.NBLK, pl.NPC, pl.GMAX
    G, G_lo, G_hi = pl.G, pl.G_lo, pl.G_hi
    SLABP = NBLK * P                      # padded slab rows (6272)
    IDXW = sum(g * 8 for g in G)
    DRW = sum(G)
    ROWW = sum(g * P for g in G)
    # the SWDGE ring appears hard-limited to 1024 idxs (64 descs/lane) per
    # call regardless of scratch size — keep calls at <=1024 idxs (gch<=8)
    assert gch * P <= 1024
    assert scratch // 16 // queues >= gch * P

    nc = bacc.Bacc("TRN2", target_bir_lowering=False, debug=False,
                   dynamic_dma_scratch_size=scratch, num_swdge_queues=queues)
    dp = lambda name, shape, dt=F32, out=False: nc.declare_dram_parameter(
        name, list(shape), dt, isOutput=out).ap()

    xl1 = dp("xl1", [pl.N, D], BF16)
    xr1_loc = dp("xr1_loc", [SLABP, D], BF16)
    idx_cat = dp("idx_cat", [P, IDXW], I16)
    drt_cat = dp("drt_cat", [P, DRW], BF16)
    drrow_cat = dp("drrow_cat", [1, ROWW], BF16)
    iota_row_p = dp("iota_row", [P, P], BF16)
    iota_col_p = dp("iota_col", [P, 1])
    ones_p = dp("ones_t", [1, P], BF16)
    ident_p = dp("ident", [P, P], BF16)
    att1_p = dp("att1_t", [P, D], BF16)
    att2_p = dp("att2_t", [P, D], BF16)
    bias1_p = dp("bias1", [P, D])
    bias2_p = dp("bias2", [P, D])
    W2l_p = dp("W2l", [D, D], BF16)
    W2r_p = dp("W2r", [D, D], BF16)
    out_p = dp("out", [NPC, D], out=True)

    xl2_slab = nc.dram_tensor("xl2_slab", [NPC, D], BF16).ap()
    xl2_full = nc.dram_tensor("xl2_full", [pl.N, D], BF16,
                              addr_space="Shared").ap()
    xr2_loc = nc.dram_tensor("xr2_loc", [SLABP, D], BF16).ap()

    # per-block column offsets into the concatenated per-core arrays
    idx_off = np.cumsum([0] + [g * 8 for g in G])
    drt_off = np.cumsum([0] + list(G))
    drrow_off = np.cumsum([0] + [g * P for g in G])

    with TileContext(nc) as tc:
        nc.gpsimd.load_library(mlp)
        with (
            tc.tile_pool(name="const", bufs=1) as cpool,
            tc.tile_pool(name="stream", bufs=3) as spool,
            tc.tile_pool(name="work", bufs=2) as wpool,
            tc.tile_pool(name="small", bufs=3) as smpool,
            tc.tile_pool(name="psA", bufs=1, space="PSUM") as psA,
            tc.tile_pool(name="psB", bufs=2, space="PSUM") as psB,
            tc.tile_pool(name="psC", bufs=1, space="PSUM") as psC,
            tc.tile_pool(name="psD", bufs=1, space="PSUM") as psD,
        ):
            def cload(name, ap_in, shape, dt=F32):
                t = cpool.tile(shape, dt, tag=name)
                nc.sync.dma_start(out=t[:], in_=ap_in)
                return t

            iota_row_c = cload("c_iotar", iota_row_p[:, :], [P, P], BF16)
            iota_col_c = cload("c_iotac", iota_col_p[:, :], [P, 1])
            ones_c = cload("c_ones", ones_p[:, :], [1, P], BF16)
            ident_c = cload("c_ident", ident_p[:, :], [P, P], BF16)
            att1_c = cload("c_att1", att1_p[:, :], [P, D], BF16)
            att2_c = cload("c_att2", att2_p[:, :], [P, D], BF16)
            bias1_c = cload("c_bias1", bias1_p[:, :], [P, D])
            bias2_c = cload("c_bias2", bias2_p[:, :], [P, D])
            W2l_c = cload("c_w2l", W2l_p[:, :], [D, D], BF16)
            W2r_c = cload("c_w2r", W2r_p[:, :], [D, D], BF16)
            alpha_c = cpool.tile([P, 1], F32)
            nc.vector.memset(alpha_c[:], SLOPE)

            qctr = [0]

            def edge_layer(tab, xr_loc_ap, att_c, bias_c, layer1):
                for b in range(NBLK):
                    g, glo, ghi = G[b], G_lo[b], G_hi[b]
                    nreal = min(NPC - b * P, P)

                    idx_t = spool.tile([P, GMAX * 8], I16, tag="idx")
                    nc.sync.dma_start(
                        out=idx_t[:, 0:g * 8],
                        in_=idx_cat[:, idx_off[b]:idx_off[b] + g * 8])
                    drt_t = spool.tile([P, GMAX], BF16, tag="drt")
                    nc.sync.dma_start(
                        out=drt_t[:, 0:g],
                        in_=drt_cat[:, drt_off[b]:drt_off[b] + g])
                    drrow_t = spool.tile([1, GMAX * P], BF16, tag="drrow")
                    nc.sync.dma_start(
                        out=drrow_t[:, 0:g * P],
                        in_=drrow_cat[:, drrow_off[b]:drrow_off[b] + g * P])
                    xrb_t = smpool.tile([P, D], BF16, tag="xrb")
                    nc.sync.dma_start(out=xrb_t[:],
                                      in_=xr_loc_ap[b * P:(b + 1) * P, :])

                    # ST[j, e] = (dst_rel[e] == j): broadcast dr_row down the
                    # partitions via a K=1 PE matmul, then compare with the
                    # per-partition iota constant.
                    drT_t = wpool.tile([P, GMAX * P], BF16, tag="drT")
                    for c0 in range(0, g * P, 512):
                        c1 = min(c0 + 512, g * P)
                        ps_dr = psA.tile([P, 512], F32, tag="psdr")
                        nc.tensor.matmul(out=ps_dr[:, 0:c1 - c0],
                                         lhsT=ones_c[:, :],
                                         rhs=drrow_t[:, c0:c1],
                                         start=True, stop=True)
                        nc.scalar.copy(out=drT_t[:, c0:c1],
                                       in_=ps_dr[:, 0:c1 - c0])
                    ST_t = wpool.tile([P, GMAX * P], BF16, tag="ST")
                    nc.vector.tensor_scalar(
                        out=ST_t[:, 0:g * P], in0=drT_t[:, 0:g * P],
                        scalar1=iota_col_c[:, :], scalar2=None,
                        op0=mybir.AluOpType.is_equal)
                    S_t = wpool.tile([P, GMAX, P], BF16, tag="S")
                    nc.vector.tensor_tensor(
                        out=S_t[:, 0:g, :],
                        in0=iota_row_c[:].unsqueeze(1).to_broadcast([P, g, P]),
                        in1=drt_t[:, 0:g].unsqueeze(2).to_broadcast([P, g, P]),
                        op=mybir.AluOpType.is_equal)

                    # A-side gather: xl[src] (bf16, 256B/edge); calls are
                    # chunked to stay within the per-queue descriptor ring.
                    za_t = wpool.tile([P, GMAX, D], BF16, tag="za")
                    for gn, g0, t0, t1 in ((glo, 0, 0, pl.SPLIT),
                                           (ghi, glo, pl.SPLIT, pl.N)):
                        for c0 in range(0, gn, gch):
                            cn = min(gch, gn - c0)
                            q0 = g0 + c0
                            nc.gpsimd.dma_gather(
                                out_ap=za_t[:, q0:q0 + cn, :],
                                in_ap=tab[t0:t1, :],
                                idxs_ap=idx_t[:, q0 * 8:(q0 + cn) * 8],
                                num_idxs=cn * P, num_idxs_reg=cn * P,
                                elem_size=D, queue_num=qctr[0] % queues)
                            qctr[0] += 1

                    # zb = ST_g @ xrb per group (PSUM), z = za + zb (bf16)
                    z_t = wpool.tile([P, GMAX, D], BF16, tag="z")
                    for c8 in range(0, g, 8):
                        n8 = min(8, g - c8)
                        ps_zb = psB.tile([P, 8, D], F32, tag="pszb")
                        for i in range(n8):
                            gi = c8 + i
                            nc.tensor.matmul(
                                out=ps_zb[:, i, :],
                                lhsT=ST_t[:, gi * P:(gi + 1) * P],
                                rhs=xrb_t[:], start=True, stop=True)
                        nc.vector.tensor_tensor(
                            out=z_t[:, c8:c8 + n8, :],
                            in0=za_t[:, c8:c8 + n8, :],
                            in1=ps_zb[:, 0:n8, :], op=mybir.AluOpType.add)

                    # LeakyReLU then att-dot (in place on z)
                    nc.scalar.activation(out=z_t[:, 0:g, :],
                                         in_=z_t[:, 0:g, :],
                                         func=mybir.ActivationFunctionType.Prelu,
                                         alpha=alpha_c[:, :])
                    nc.vector.tensor_tensor(
                        out=z_t[:, 0:g, :], in0=z_t[:, 0:g, :],
                        in1=att_c[:].unsqueeze(1).to_broadcast([P, g, D]),
                        op=mybir.AluOpType.mult)
                    # Sum over C via a strided halving tree of packed bf16
                    # adds (2x DVE mode) — cheaper than tensor_reduce (1x).
                    e_t = smpool.tile([P, GMAX, H], BF16, tag="e")
                    zv = z_t[:, 0:g, :].rearrange("p g (h c) -> p g h c", h=H)
                    with nc.allow_low_precision(
                            reason="bf16 e-scores; abs tol 2e-2 validated"):
                        for w in (16, 8, 4, 2):
                            nc.vector.tensor_tensor(
                                out=zv[:, :, :, 0:w], in0=zv[:, :, :, 0:w],
                                in1=zv[:, :, :, w:2 * w],
                                op=mybir.AluOpType.add)
                        nc.vector.tensor_tensor(
                            out=e_t[:, 0:g, :].unsqueeze(3),
                            in0=zv[:, :, :, 0:1],
                            in1=zv[:, :, :, 1:2], op=mybir.AluOpType.add)

                    # comb = [u | w]: w = exp(e) into cols D: (small) and
                    # expanded across C via a broadcast-input ACT exp (wex),
                    # so u = wex*za stays a packed 2x DVE mult.
                    comb_t = wpool.tile([P, GMAX, D + H], BF16, tag="comb")
                    nc.scalar.activation(out=comb_t[:, 0:g, D:D + H],
                                         in_=e_t[:, 0:g, :],
                                         func=mybir.ActivationFunctionType.Exp)
                    wex_t = wpool.tile([P, GMAX, D], BF16, tag="wex")
                    nc.scalar.activation(
                        out=wex_t[:, 0:g, :].rearrange(
                            "p g (h c) -> p g h c", h=H),
                        in_=e_t[:, 0:g, :].unsqueeze(3).to_broadcast(
                            [P, g, H, C]),
                        func=mybir.ActivationFunctionType.Exp)
                    nc.vector.tensor_tensor(
                        out=comb_t[:, 0:g, 0:D],
                        in0=za_t[:, 0:g, :],
                        in1=wex_t[:, 0:g, :],
                        op=mybir.AluOpType.mult)

                    # scatter: psum[j, D+H] += S_g^T @ comb_g
                    ps_agg = psC.tile([P, D + H], F32, tag="agg")
                    for gi in range(g):
                        nc.tensor.matmul(out=ps_agg[:],
                                         lhsT=S_t[:, gi, :],
                                         rhs=comb_t[:, gi, :],
                                         start=(gi == 0), stop=(gi == g - 1))

                    # epilogue
                    deps = smpool.tile([P, H], F32, tag="deps")
                    nc.vector.tensor_scalar_add(out=deps[:],
                                                in0=ps_agg[:, D:D + H],
                                                scalar1=1e-16)
                    dinv = smpool.tile([P, H], F32, tag="dinv")
                    nc.vector.reciprocal(out=dinv[:], in_=deps[:])
                    res = smpool.tile([P, D], F32, tag="res")
                    nc.vector.tensor_tensor(
                        out=res[:].rearrange("p (h c) -> p h c", h=H),
                        in0=ps_agg[:, 0:D].rearrange("p (h c) -> p h c", h=H),
                        in1=dinv[:].unsqueeze(2).to_broadcast([P, H, C]),
                        op=mybir.AluOpType.mult)
                    nc.vector.tensor_tensor(out=res[:], in0=res[:],
                                            in1=bias_c[:],
                                            op=mybir.AluOpType.add)

                    if layer1:
                        h_t = smpool.tile([P, D], BF16, tag="h")
                        nc.scalar.activation(
                            out=h_t[:], in_=res[:],
                            func=mybir.ActivationFunctionType.Relu)
                        psT = psD.tile([P, P], BF16, tag="xfm")
                        nc.tensor.transpose(out=psT[:], in_=h_t[:],
                                            identity=ident_c[:])
                        hT = smpool.tile([P, P], BF16, tag="hT")
                        nc.scalar.copy(out=hT[:], in_=psT[:])
                        for W_c, table, rows in ((W2l_c, xl2_slab, nreal),
                                                 (W2r_c, xr2_loc, P)):
                            psm = psD.tile([P, D], F32, tag="xfm2")
                            nc.tensor.matmul(out=psm[:], lhsT=hT[:],
                                             rhs=W_c[:], start=True, stop=True)
                            rt = smpool.tile([P, D], BF16, tag="xfm_o")
                            nc.scalar.copy(out=rt[:], in_=psm[:])
                            nc.sync.dma_start(
                                out=table[b * P:b * P + rows, :],
                                in_=rt[0:rows, :])
                    else:
                        h32 = smpool.tile([P, D], F32, tag="h32")
                        nc.scalar.activation(
                            out=h32[:], in_=res[:],
                            func=mybir.ActivationFunctionType.Relu)
                        nc.sync.dma_start(
                            out=out_p[b * P:b * P + nreal, :],
                            in_=h32[0:nreal, :])

            for _rep in range(repeat):
                edge_layer(xl1, xr1_loc, att1_c, bias1_c, layer1=True)
                if not skip_collective:
                    nc.gpsimd.collective_compute(
                        "AllGather", mybir.AluOpType.bypass,
                        replica_groups=[list(range(pl.NC))],
                        ins=[xl2_slab[:, :].opt()],
                        outs=[xl2_full[:, :].opt()],
                    )
                edge_layer(xl2_full, xr2_loc, att2_c, bias2_c, layer1=False)

    return nc


def make_inputs(plan, datas, x, W1_l, W1_r, att1, b1, W2_l, W2_r, att2, b2):
    pl = plan
    GMAX = pl.GMAX
    x = np.asarray(x, np.float32)
    xl1 = (x @ np.asarray(W1_l, np.float32)).astype(NPBF)
    xr1 = (x @ np.asarray(W1_r, np.float32)).astype(NPBF)

    iota_col = np.arange(P, dtype=np.float32)[:, None]
    iota_row = np.tile(np.arange(P, dtype=np.float32)[None, :],
                       (P, 1)).astype(NPBF)
    ones_t = np.ones((1, P), NPBF)
    ident = np.eye(P, dtype=np.float32).astype(NPBF)
    att1_t = np.tile(np.asarray(att1, np.float32).reshape(1, D),
                     (P, 1)).astype(NPBF)
    att2_t = np.tile(np.asarray(att2, np.float32).reshape(1, D),
                     (P, 1)).astype(NPBF)
    bias1_t = np.tile(np.asarray(b1, np.float32).reshape(1, D), (P, 1))
    bias2_t = np.tile(np.asarray(b2, np.float32).reshape(1, D), (P, 1))

    SLABP = pl.NBLK * P
    in_maps = []
    for k in range(pl.NC):
        xr1_loc = np.zeros((SLABP, D), NPBF)
        xr1_loc[:pl.NPC] = xr1[k * pl.NPC:(k + 1) * pl.NPC]
        in_maps.append(dict(
            xl1=xl1, xr1_loc=xr1_loc,
            idx_cat=datas[k]["idx_cat"],
            drt_cat=datas[k]["drt_cat"],
            drrow_cat=datas[k]["drrow_cat"],
            iota_col=iota_col, iota_row=iota_row, ones_t=ones_t, ident=ident,
            att1_t=att1_t, att2_t=att2_t,
            bias1=bias1_t.astype(np.float32), bias2=bias2_t.astype(np.float32),
            W2l=np.asarray(W2_l, np.float32).astype(NPBF),
            W2r=np.asarray(W2_r, np.float32).astype(NPBF),
        ))
    return in_maps


def assemble_output(plan, results):
    out = np.zeros((plan.N, D), np.float32)
    for k in range(plan.NC):
        out[k * plan.NPC:(k + 1) * plan.NPC] = results[k]["out"]
    return out


def kernel(x, edge_index, W1_l, W1_r, att1, b1, W2_l, W2_r, att2, b2):
    x = np.ascontiguousarray(np.asarray(x, np.float32))
    plan, datas = preprocess(x, np.asarray(edge_index), NC=8)
    nc = build_kernel(plan)
    nc.compile()
    in_maps = make_inputs(plan, datas, x, W1_l, W1_r, att1, b1,
                          W2_l, W2_r, att2, b2)
    res = run_bass_kernel_spmd(nc, in_maps, core_ids=list(range(8)))
    return assemble_output(plan, res.results)


# revision 32
# speedup vs baseline: 3.9026x; 3.9026x over previous
"""GATv2 2-layer GNN message-passing kernel for Trainium2, 8-core SPMD (v2).

Contract: kernel(**inputs) takes FULL unsharded inputs and returns the FULL
[50000, 128] float32 output.

Strategy (edge/data parallel, dst-range sharded, descriptor-minimized):
- Host: append self-loops, sort edges by dst; each core owns 6250 dst nodes
  = 49 blocks of 128. Per block, edges are split by src-half (int16 gather
  index limit) and padded to 128-edge groups with per-block group counts.
- All tables and per-edge compute are bf16 (fp32 PSUM accumulation);
  absmax-rel error ~5e-3 vs fp32 (gate 2e-2).
- Only ONE dma_gather stream per edge (xl[src], 256B descriptors) spread
  over 4 SWDGE queues with a 64KB descriptor ring. xr[dst] is NOT gathered:
  dst is block-local, so xr rows are reconstructed on the PE as
  zb = ST_g @ xrb  (ST = transposed one-hot of dst_rel built on-chip).
- Per-edge pipeline: z = za+zb (DVE), LeakyReLU (ACT), att-dot (DVE
  mult+reduce), w = exp (ACT, written into comb), u = w*za (DVE, into comb),
  then one PE matmul chain scatters S^T @ [u|w] into the block PSUM.
  out = relu(psum_u / (psum_w+eps) + bias).
- Layer 2 reuses the SAME gather indices: the AllGather writes xl2 in global
  node order ([50000,128] bf16), and xr2 stays core-local.
"""
import sys
sys.path.insert(0, '/opt/trn_rl_repo')
import numpy as np
from dataclasses import dataclass, field

import concourse.bass as bass
import concourse.bacc as bacc
import concourse.mybir as mybir
from concourse.tile import TileContext
from concourse.library_config import mlp
from concourse.bass_utils import run_bass_kernel_spmd

P = 128
H, C = 4, 32
D = H * C          # 128
SLOPE = 0.2
F32 = mybir.dt.float32
BF16 = mybir.dt.bfloat16
I16 = mybir.dt.int16
NPBF = mybir.dt.np(BF16)


@dataclass
class Plan:
    N: int
    NC: int
    NPC: int            # nodes per core (6250)
    NBLK: int           # blocks per core (49)
    SPLIT: int          # lo/hi table split (25000)
    G_lo: list = field(default_factory=list)   # per-block lo group count
    G_hi: list = field(default_factory=list)   # per-block hi group count

    @property
    def G(self):
        return [a + b for a, b in zip(self.G_lo, self.G_hi)]

    @property
    def GMAX(self):
        return max(self.G)


def wrap_idx(flat):
    """[n] int -> dma_gather SBUF idx layout [128, n//16]."""
    n = flat.shape[0]
    assert n % 16 == 0
    w = flat.reshape(n // 16, 16).T
    return np.tile(w, (8, 1)).astype(np.int16)


def preprocess(x, edge_index, NC=8):
    N = x.shape[0]
    NPC = N // NC
    NBLK = (NPC + P - 1) // P
    SPLIT = N // 2

    loop = np.arange(N, dtype=np.int64)
    src = np.concatenate([np.asarray(edge_index[0]), loop]).astype(np.int64)
    dst = np.concatenate([np.asarray(edge_index[1]), loop]).astype(np.int64)
    order = np.argsort(dst, kind='stable')
    src = src[order].astype(np.int32)
    dst = dst[order].astype(np.int32)
    core_bounds = np.searchsorted(dst, np.arange(NC + 1) * NPC)

    per_core = []
    lo_counts = np.zeros((NC, NBLK), np.int64)
    hi_counts = np.zeros((NC, NBLK), np.int64)
    for k in range(NC):
        a, b = core_bounds[k], core_bounds[k + 1]
        s_k = src[a:b]
        d_k = dst[a:b] - k * NPC
        blk = d_k // P
        is_lo = s_k < SPLIT
        lo_counts[k] = np.bincount(blk[is_lo], minlength=NBLK)
        hi_counts[k] = np.bincount(blk[~is_lo], minlength=NBLK)
        per_core.append((s_k, d_k, blk, is_lo))

    plan = Plan(N=N, NC=NC, NPC=NPC, NBLK=NBLK, SPLIT=SPLIT)
    plan.G_lo = [int(v) for v in
                 (lo_counts.max(axis=0) + P - 1) // P]
    plan.G_hi = [int(v) for v in
                 (hi_counts.max(axis=0) + P - 1) // P]
    G = plan.G

    datas = []
    for k in range(NC):
        s_k, d_k, blk, is_lo = per_core[k]
        idx_cols, drt_cols, drrow_cols = [], [], []
        for b in range(NBLK):
            in_b = blk == b
            dr_flat = np.full(G[b] * P, -1.0, np.float32)
            wrapped = []
            for side, gn, off in ((True, plan.G_lo[b], 0),
                                  (False, plan.G_hi[b], plan.G_lo[b] * P)):
                flat = np.zeros(gn * P, np.int16)
                sel = in_b & (is_lo == side)
                ss = s_k[sel]
                n = ss.shape[0]
                assert n <= gn * P, (k, b, side, n, gn)
                flat[:n] = ss if side else ss - SPLIT
                dr_flat[off:off + n] = d_k[sel] - b * P
                if gn:
                    wrapped.append(wrap_idx(flat))
            idx_cols.append(np.concatenate(wrapped, axis=1))
            drt_cols.append(dr_flat.reshape(G[b], P).T)   # [128, G]
            drrow_cols.append(dr_flat.reshape(1, G[b] * P))
        datas.append(dict(
            idx_cat=np.ascontiguousarray(np.concatenate(idx_cols, axis=1)),
            drt_cat=np.ascontiguousarray(
                np.concatenate(drt_cols, axis=1)).astype(NPBF),
            drrow_cat=np.ascontiguousarray(
                np.concatenate(drrow_cols, axis=1)).astype(NPBF),
        ))
    return plan, datas


def build_kernel(plan, repeat=1, skip_collective=False, queues=1,
                 scratch=16384, gch=8):
    pl = plan
    NBLK, NPC, GMAX = pl.NBLK, pl.NPC, pl.GMAX
    G, G_lo, G_hi = pl.G, pl.G_lo, pl.G_hi
    SLABP = NBLK * P                      # padded slab rows (6272)
    IDXW = sum(g * 8 for g in G)
    DRW = sum(G)
    ROWW = sum(g * P for g in G)
    # the SWDGE ring appears hard-limited to 1024 idxs (64 descs/lane) per
    # call regardless of scratch size — keep calls at <=1024 idxs (gch<=8)
    assert gch * P <= 1024
    assert scratch // 16 // queues >= gch * P

    nc = bacc.Bacc("TRN2", target_bir_lowering=False, debug=False,
                   dynamic_dma_scratch_size=scratch, num_swdge_queues=queues)
    dp = lambda name, shape, dt=F32, out=False: nc.declare_dram_parameter(
        name, list(shape), dt, isOutput=out).ap()

    xl1 = dp("xl1", [pl.N, D], BF16)
    xr1_loc = dp("xr1_loc", [SLABP, D], BF16)
    idx_cat = dp("idx_cat", [P, IDXW], I16)
    drt_cat = dp("drt_cat", [P, DRW], BF16)
    drrow_cat = dp("drrow_cat", [1, ROWW], BF16)
    iota_row_p = dp("iota_row", [P, P], BF16)
    iota_col_p = dp("iota_col", [P, 1])
    ones_p = dp("ones_t", [1, P], BF16)
    ident_p = dp("ident", [P, P], BF16)
    att1_p = dp("att1_t", [P, D], BF16)
    att2_p = dp("att2_t", [P, D], BF16)
    bias1_p = dp("bias1", [P, D])
    bias2_p = dp("bias2", [P, D])
    W2l_p = dp("W2l", [D, D], BF16)
    W2r_p = dp("W2r", [D, D], BF16)
    out_p = dp("out", [NPC, D], out=True)

    xl2_slab = nc.dram_tensor("xl2_slab", [NPC, D], BF16).ap()
    xl2_full = nc.dram_tensor("xl2_full", [pl.N, D], BF16,
                              addr_space="Shared").ap()
    xr2_loc = nc.dram_tensor("xr2_loc", [SLABP, D], BF16).ap()

    # per-block column offsets into the concatenated per-core arrays
    idx_off = np.cumsum([0] + [g * 8 for g in G])
    drt_off = np.cumsum([0] + list(G))
    drrow_off = np.cumsum([0] + [g * P for g in G])

    with TileContext(nc) as tc:
        nc.gpsimd.load_library(mlp)
        with (
            tc.tile_pool(name="const", bufs=1) as cpool,
            tc.tile_pool(name="stream", bufs=3) as spool,
            tc.tile_pool(name="work", bufs=2) as wpool,
            tc.tile_pool(name="small", bufs=3) as smpool,
            tc.tile_pool(name="psA", bufs=1, space="PSUM") as psA,
            tc.tile_pool(name="psB", bufs=2, space="PSUM") as psB,
            tc.tile_pool(name="psC", bufs=1, space="PSUM") as psC,
            tc.tile_pool(name="psD", bufs=1, space="PSUM") as psD,
        ):
            def cload(name, ap_in, shape, dt=F32):
                t = cpool.tile(shape, dt, tag=name)
                nc.sync.dma_start(out=t[:], in_=ap_in)
                return t

            iota_row_c = cload("c_iotar", iota_row_p[:, :], [P, P], BF16)
            iota_col_c = cload("c_iotac", iota_col_p[:, :], [P, 1])
            ones_c = cload("c_ones", ones_p[:, :], [1, P], BF16)
            ident_c = cload("c_ident", ident_p[:, :], [P, P], BF16)
            att1_c = cload("c_att1", att1_p[:, :], [P, D], BF16)
            att2_c = cload("c_att2", att2_p[:, :], [P, D], BF16)
            bias1_c = cload("c_bias1", bias1_p[:, :], [P, D])
            bias2_c = cload("c_bias2", bias2_p[:, :], [P, D])
            W2l_c = cload("c_w2l", W2l_p[:, :], [D, D], BF16)
            W2r_c = cload("c_w2r", W2r_p[:, :], [D, D], BF16)
            alpha_c = cpool.tile([P, 1], F32)
            nc.vector.memset(alpha_c[:], SLOPE)

            qctr = [0]

            def edge_layer(tab, xr_loc_ap, att_c, bias_c, layer1):
                for b in range(NBLK):
                    g, glo, ghi = G[b], G_lo[b], G_hi[b]
                    nreal = min(NPC - b * P, P)

                    idx_t = spool.tile([P, GMAX * 8], I16, tag="idx")
                    nc.sync.dma_start(
                        out=idx_t[:, 0:g * 8],
                        in_=idx_cat[:, idx_off[b]:idx_off[b] + g * 8])
                    drt_t = spool.tile([P, GMAX], BF16, tag="drt")
                    nc.sync.dma_start(
                        out=drt_t[:, 0:g],
                        in_=drt_cat[:, drt_off[b]:drt_off[b] + g])
                    xrb_t = smpool.tile([P, D], BF16, tag="xrb")
                    nc.sync.dma_start(out=xrb_t[:],
                                      in_=xr_loc_ap[b * P:(b + 1) * P, :])

                    # S[e, g, j] = (dst_rel[e] == j); ST[j, g, e] is its
                    # per-group transpose, built by one xbar DMA transpose.
                    S_t = wpool.tile([P, GMAX, P], BF16, tag="S", bufs=3)
                    nc.vector.tensor_tensor(
                        out=S_t[:, 0:g, :],
                        in0=iota_row_c[:].unsqueeze(1).to_broadcast([P, g, P]),
                        in1=drt_t[:, 0:g].unsqueeze(2).to_broadcast([P, g, P]),
                        op=mybir.AluOpType.is_equal)
                    ST_t = wpool.tile([P, GMAX, P], BF16, tag="ST")
                    nc.sync.dma_start_transpose(
                        out=ST_t[:, 0:g, :],
                        in_=S_t[:, 0:g, :].rearrange("p g j -> p (g j)"))

                    # A-side gather: xl[src] (bf16, 256B/edge); calls are
                    # chunked to stay within the per-queue descriptor ring.
                    za_t = wpool.tile([P, GMAX, D], BF16, tag="za", bufs=3)
                    for gn, g0, t0, t1 in ((glo, 0, 0, pl.SPLIT),
                                           (ghi, glo, pl.SPLIT, pl.N)):
                        for c0 in range(0, gn, gch):
                            cn = min(gch, gn - c0)
                            q0 = g0 + c0
                            nc.gpsimd.dma_gather(
                                out_ap=za_t[:, q0:q0 + cn, :],
                                in_ap=tab[t0:t1, :],
                                idxs_ap=idx_t[:, q0 * 8:(q0 + cn) * 8],
                                num_idxs=cn * P, num_idxs_reg=cn * P,
                                elem_size=D, queue_num=qctr[0] % queues)
                            qctr[0] += 1

                    # zb = ST_g @ xrb per group (PSUM) -> ACT copy to bf16;
                    # then ONE packed 2x DVE add z = za + zb.
                    zbS_t = wpool.tile([P, GMAX, D], BF16, tag="zbS")
                    for c8 in range(0, g, 8):
                        n8 = min(8, g - c8)
                        ps_zb = psB.tile([P, 8, D], F32, tag="pszb")
                        for i in range(n8):
                            gi = c8 + i
                            nc.tensor.matmul(
                                out=ps_zb[:, i, :],
                                lhsT=ST_t[:, gi, :],
                                rhs=xrb_t[:], start=True, stop=True)
                        nc.scalar.copy(out=zbS_t[:, c8:c8 + n8, :],
                                       in_=ps_zb[:, 0:n8, :])
                    z_t = wpool.tile([P, GMAX, D], BF16, tag="z")
                    nc.vector.tensor_tensor(
                        out=z_t[:, 0:g, :], in0=za_t[:, 0:g, :],
                        in1=zbS_t[:, 0:g, :], op=mybir.AluOpType.add)

                    # LeakyReLU -> att-mult -> C-sum tree, processed in two
                    # group-halves so ACT (prelu) and DVE (mult/tree) overlap.
                    e_t = smpool.tile([P, GMAX, H], BF16, tag="e")
                    halves = [(0, g // 2), (g // 2, g)] if g > 1 else [(0, g)]
                    for h0, h1 in halves:
                        hw_ = h1 - h0
                        if not hw_:
                            continue
                        nc.scalar.activation(
                            out=z_t[:, h0:h1, :], in_=z_t[:, h0:h1, :],
                            func=mybir.ActivationFunctionType.Prelu,
                            alpha=alpha_c[:, :])
                        nc.vector.tensor_tensor(
                            out=z_t[:, h0:h1, :], in0=z_t[:, h0:h1, :],
                            in1=att_c[:].unsqueeze(1).to_broadcast(
                                [P, hw_, D]),
                            op=mybir.AluOpType.mult)
                        zv = z_t[:, h0:h1, :].rearrange(
                            "p g (h c) -> p g h c", h=H)
                        with nc.allow_low_precision(
                                reason="bf16 e-scores; abs tol 2e-2 ok"):
                            for w in (16, 8, 4, 2):
                                nc.vector.tensor_tensor(
                                    out=zv[:, :, :, 0:w], in0=zv[:, :, :, 0:w],
                                    in1=zv[:, :, :, w:2 * w],
                                    op=mybir.AluOpType.add)
                            nc.vector.tensor_tensor(
                                out=e_t[:, h0:h1, :].unsqueeze(3),
                                in0=zv[:, :, :, 0:1],
                                in1=zv[:, :, :, 1:2], op=mybir.AluOpType.add)

                    # comb = [u | w]: w = exp(e) into cols D: (small) and
                    # expanded across C via a broadcast-input ACT exp (wex),
                    # so u = wex*za stays a packed 2x DVE mult.
                    comb_t = wpool.tile([P, GMAX, D + H], BF16, tag="comb")
                    nc.scalar.activation(out=comb_t[:, 0:g, D:D + H],
                                         in_=e_t[:, 0:g, :],
                                         func=mybir.ActivationFunctionType.Exp)
                    wex_t = wpool.tile([P, GMAX, D], BF16, tag="wex")
                    nc.scalar.activation(
                        out=wex_t[:, 0:g, :].rearrange(
                            "p g (h c) -> p g h c", h=H),
                        in_=e_t[:, 0:g, :].unsqueeze(3).to_broadcast(
                            [P, g, H, C]),
                        func=mybir.ActivationFunctionType.Exp)
                    nc.vector.tensor_tensor(
                        out=comb_t[:, 0:g, 0:D],
                        in0=za_t[:, 0:g, :],
                        in1=wex_t[:, 0:g, :],
                        op=mybir.AluOpType.mult)

                    # scatter: psum[j, D+H] += S_g^T @ comb_g
                    ps_agg = psC.tile([P, D + H], F32, tag="agg")
                    for gi in range(g):
                        nc.tensor.matmul(out=ps_agg[:],
                                         lhsT=S_t[:, gi, :],
                                         rhs=comb_t[:, gi, :],
                                         start=(gi == 0), stop=(gi == g - 1))

                    # epilogue
                    deps = smpool.tile([P, H], F32, tag="deps")
                    nc.vector.tensor_scalar_add(out=deps[:],
                                                in0=ps_agg[:, D:D + H],
                                                scalar1=1e-16)
                    dinv = smpool.tile([P, H], F32, tag="dinv")
                    nc.vector.reciprocal(out=dinv[:], in_=deps[:])
                    res = smpool.tile([P, D], F32, tag="res")
                    nc.vector.tensor_tensor(
                        out=res[:].rearrange("p (h c) -> p h c", h=H),
                        in0=ps_agg[:, 0:D].rearrange("p (h c) -> p h c", h=H),
                        in1=dinv[:].unsqueeze(2).to_broadcast([P, H, C]),
                        op=mybir.AluOpType.mult)
                    nc.vector.tensor_tensor(out=res[:], in0=res[:],
                                            in1=bias_c[:],
                                            op=mybir.AluOpType.add)

                    if layer1:
                        h_t = smpool.tile([P, D], BF16, tag="h")
                        nc.scalar.activation(
                            out=h_t[:], in_=res[:],
                            func=mybir.ActivationFunctionType.Relu)
                        psT = psD.tile([P, P], BF16, tag="xfm")
                        nc.tensor.transpose(out=psT[:], in_=h_t[:],
                                            identity=ident_c[:])
                        hT = smpool.tile([P, P], BF16, tag="hT")
                        nc.scalar.copy(out=hT[:], in_=psT[:])
                        for W_c, table, rows in ((W2l_c, xl2_slab, nreal),
                                                 (W2r_c, xr2_loc, P)):
                            psm = psD.tile([P, D], F32, tag="xfm2")
                            nc.tensor.matmul(out=psm[:], lhsT=hT[:],
                                             rhs=W_c[:], start=True, stop=True)
                            rt = smpool.tile([P, D], BF16, tag="xfm_o")
                            nc.scalar.copy(out=rt[:], in_=psm[:])
                            nc.sync.dma_start(
                                out=table[b * P:b * P + rows, :],
                                in_=rt[0:rows, :])
                    else:
                        h32 = smpool.tile([P, D], F32, tag="h32")
                        nc.scalar.activation(
                            out=h32[:], in_=res[:],
                            func=mybir.ActivationFunctionType.Relu)
                        nc.sync.dma_start(
                            out=out_p[b * P:b * P + nreal, :],
                            in_=h32[0:nreal, :])

            for _rep in range(repeat):
                edge_layer(xl1, xr1_loc, att1_c, bias1_c, layer1=True)
                if not skip_collective:
                    nc.gpsimd.collective_compute(
                        "AllGather", mybir.AluOpType.bypass,
                        replica_groups=[list(range(pl.NC))],
                        ins=[xl2_slab[:, :].opt()],
                        outs=[xl2_full[:, :].opt()],
                    )
                edge_layer(xl2_full, xr2_loc, att2_c, bias2_c, layer1=False)

    return nc


def make_inputs(plan, datas, x, W1_l, W1_r, att1, b1, W2_l, W2_r, att2, b2):
    pl = plan
    GMAX = pl.GMAX
    x = np.asarray(x, np.float32)
    xl1 = (x @ np.asarray(W1_l, np.float32)).astype(NPBF)
    xr1 = (x @ np.asarray(W1_r, np.float32)).astype(NPBF)

    iota_col = np.arange(P, dtype=np.float32)[:, None]
    iota_row = np.tile(np.arange(P, dtype=np.float32)[None, :],
                       (P, 1)).astype(NPBF)
    ones_t = np.ones((1, P), NPBF)
    ident = np.eye(P, dtype=np.float32).astype(NPBF)
    att1_t = np.tile(np.asarray(att1, np.float32).reshape(1, D),
                     (P, 1)).astype(NPBF)
    att2_t = np.tile(np.asarray(att2, np.float32).reshape(1, D),
                     (P, 1)).astype(NPBF)
    bias1_t = np.tile(np.asarray(b1, np.float32).reshape(1, D), (P, 1))
    bias2_t = np.tile(np.asarray(b2, np.float32).reshape(1, D), (P, 1))

    SLABP = pl.NBLK * P
    in_maps = []
    for k in range(pl.NC):
        xr1_loc = np.zeros((SLABP, D), NPBF)
        xr1_loc[:pl.NPC] = xr1[k * pl.NPC:(k + 1) * pl.NPC]
        in_maps.append(dict(
            xl1=xl1, xr1_loc=xr1_loc,
            idx_cat=datas[k]["idx_cat"],
            drt_cat=datas[k]["drt_cat"],
            drrow_cat=datas[k]["drrow_cat"],
            iota_col=iota_col, iota_row=iota_row, ones_t=ones_t, ident=ident,
            att1_t=att1_t, att2_t=att2_t,
            bias1=bias1_t.astype(np.float32), bias2=bias2_t.astype(np.float32),
            W2l=np.asarray(W2_l, np.float32).astype(NPBF),
            W2r=np.asarray(W2_r, np.float32).astype(NPBF),
        ))
    return in_maps


def assemble_output(plan, results):
    out = np.zeros((plan.N, D), np.float32)
    for k in range(plan.NC):
        out[k * plan.NPC:(k + 1) * plan.NPC] = results[k]["out"]
    return out


def kernel(x, edge_index, W1_l, W1_r, att1, b1, W2_l, W2_r, att2, b2):
    x = np.ascontiguousarray(np.asarray(x, np.float32))
    plan, datas = preprocess(x, np.asarray(edge_index), NC=8)
    nc = build_kernel(plan)
    nc.compile()
    in_maps = make_inputs(plan, datas, x, W1_l, W1_r, att1, b1,
                          W2_l, W2_r, att2, b2)
    res = run_bass_kernel_spmd(nc, in_maps, core_ids=list(range(8)))
    return assemble_output(plan, res.results)


# revision 35
# speedup vs baseline: 7.9084x; 2.0264x over previous
"""GATv2 2-layer GNN message-passing kernel for Trainium2, 8-core SPMD (v2).

Contract: kernel(**inputs) takes FULL unsharded inputs and returns the FULL
[50000, 128] float32 output.

Strategy (edge/data parallel, dst-range sharded, descriptor-minimized):
- Host: append self-loops, sort edges by dst; each core owns 6250 dst nodes
  = 49 blocks of 128. Per block, edges are split by src-half (int16 gather
  index limit) and padded to 128-edge groups with per-block group counts.
- All tables and per-edge compute are bf16 (fp32 PSUM accumulation);
  absmax-rel error ~5e-3 vs fp32 (gate 2e-2).
- Only ONE dma_gather stream per edge (xl[src], 256B descriptors) spread
  over 4 SWDGE queues with a 64KB descriptor ring. xr[dst] is NOT gathered:
  dst is block-local, so xr rows are reconstructed on the PE as
  zb = ST_g @ xrb  (ST = transposed one-hot of dst_rel built on-chip).
- Per-edge pipeline: z = za+zb (DVE), LeakyReLU (ACT), att-dot (DVE
  mult+reduce), w = exp (ACT, written into comb), u = w*za (DVE, into comb),
  then one PE matmul chain scatters S^T @ [u|w] into the block PSUM.
  out = relu(psum_u / (psum_w+eps) + bias).
- Layer 2 reuses the SAME gather indices: the AllGather writes xl2 in global
  node order ([50000,128] bf16), and xr2 stays core-local.
"""
import sys
sys.path.insert(0, '/opt/trn_rl_repo')
import numpy as np
from dataclasses import dataclass, field

import concourse.bass as bass
import concourse.bacc as bacc
import concourse.mybir as mybir
from concourse.tile import TileContext
from concourse.library_config import mlp
from concourse.bass_utils import run_bass_kernel_spmd

P = 128
H, C = 4, 32
D = H * C          # 128
SLOPE = 0.2
F32 = mybir.dt.float32
BF16 = mybir.dt.bfloat16
I16 = mybir.dt.int16
NPBF = mybir.dt.np(BF16)


@dataclass
class Plan:
    N: int
    NC: int
    NPC: int            # nodes per core (6250)
    NBLK: int           # blocks per core (49)
    SPLIT: int          # lo/hi table split (25000)
    G_lo: list = field(default_factory=list)   # per-block lo group count
    G_hi: list = field(default_factory=list)   # per-block hi group count

    @property
    def G(self):
        return [a + b for a, b in zip(self.G_lo, self.G_hi)]

    @property
    def GMAX(self):
        return max(self.G)


def wrap_idx(flat):
    """[n] int -> dma_gather SBUF idx layout [128, n//16]."""
    n = flat.shape[0]
    assert n % 16 == 0
    w = flat.reshape(n // 16, 16).T
    return np.tile(w, (8, 1)).astype(np.int16)


def preprocess(x, edge_index, NC=8):
    N = x.shape[0]
    NPC = N // NC
    NBLK = (NPC + P - 1) // P
    SPLIT = N // 2

    loop = np.arange(N, dtype=np.int64)
    src = np.concatenate([np.asarray(edge_index[0]), loop]).astype(np.int64)
    dst = np.concatenate([np.asarray(edge_index[1]), loop]).astype(np.int64)
    order = np.argsort(dst, kind='stable')
    src = src[order].astype(np.int32)
    dst = dst[order].astype(np.int32)
    core_bounds = np.searchsorted(dst, np.arange(NC + 1) * NPC)

    per_core = []
    lo_counts = np.zeros((NC, NBLK), np.int64)
    hi_counts = np.zeros((NC, NBLK), np.int64)
    for k in range(NC):
        a, b = core_bounds[k], core_bounds[k + 1]
        s_k = src[a:b]
        d_k = dst[a:b] - k * NPC
        blk = d_k // P
        is_lo = s_k < SPLIT
        lo_counts[k] = np.bincount(blk[is_lo], minlength=NBLK)
        hi_counts[k] = np.bincount(blk[~is_lo], minlength=NBLK)
        per_core.append((s_k, d_k, blk, is_lo))

    plan = Plan(N=N, NC=NC, NPC=NPC, NBLK=NBLK, SPLIT=SPLIT)
    plan.G_lo = [int(v) for v in
                 (lo_counts.max(axis=0) + P - 1) // P]
    plan.G_hi = [int(v) for v in
                 (hi_counts.max(axis=0) + P - 1) // P]
    G = plan.G

    datas = []
    for k in range(NC):
        s_k, d_k, blk, is_lo = per_core[k]
        idx_cols, drt_cols, drrow_cols = [], [], []
        for b in range(NBLK):
            in_b = blk == b
            dr_flat = np.full(G[b] * P, -1.0, np.float32)
            wrapped = []
            for side, gn, off in ((True, plan.G_lo[b], 0),
                                  (False, plan.G_hi[b], plan.G_lo[b] * P)):
                flat = np.zeros(gn * P, np.int16)
                sel = in_b & (is_lo == side)
                ss = s_k[sel]
                n = ss.shape[0]
                assert n <= gn * P, (k, b, side, n, gn)
                flat[:n] = ss if side else ss - SPLIT
                dr_flat[off:off + n] = d_k[sel] - b * P
                if gn:
                    wrapped.append(wrap_idx(flat))
            idx_cols.append(np.concatenate(wrapped, axis=1))
            drt_cols.append(dr_flat.reshape(G[b], P).T)   # [128, G]
            drrow_cols.append(dr_flat.reshape(1, G[b] * P))
        datas.append(dict(
            idx_cat=np.ascontiguousarray(np.concatenate(idx_cols, axis=1)),
            drt_cat=np.ascontiguousarray(
                np.concatenate(drt_cols, axis=1)).astype(NPBF),
            drrow_cat=np.ascontiguousarray(
                np.concatenate(drrow_cols, axis=1)).astype(NPBF),
        ))
    return plan, datas


def build_kernel(plan, repeat=1, skip_collective=False, queues=4,
                 scratch=65536, gch=8):
    pl = plan
    NBLK, NPC, GMAX = pl.NBLK, pl.NPC, pl.GMAX
    G, G_lo, G_hi = pl.G, pl.G_lo, pl.G_hi
    SLABP = NBLK * P                      # padded slab rows (6272)
    IDXW = sum(g * 8 for g in G)
    DRW = sum(G)
    ROWW = sum(g * P for g in G)
    # the SWDGE ring appears hard-limited to 1024 idxs (64 descs/lane) per
    # call regardless of scratch size — keep calls at <=1024 idxs (gch<=8)
    assert gch * P <= 1024
    assert scratch // 16 // queues >= gch * P

    nc = bacc.Bacc("TRN2", target_bir_lowering=False, debug=False,
                   dynamic_dma_scratch_size=scratch, num_swdge_queues=queues)
    dp = lambda name, shape, dt=F32, out=False: nc.declare_dram_parameter(
        name, list(shape), dt, isOutput=out).ap()

    xl1 = dp("xl1", [pl.N, D], BF16)
    xr1_loc = dp("xr1_loc", [SLABP, D], BF16)
    idx_cat = dp("idx_cat", [P, IDXW], I16)
    drt_cat = dp("drt_cat", [P, DRW], BF16)
    drrow_cat = dp("drrow_cat", [1, ROWW], BF16)
    iota_row_p = dp("iota_row", [P, P], BF16)
    iota_col_p = dp("iota_col", [P, 1])
    ones_p = dp("ones_t", [1, P], BF16)
    ident_p = dp("ident", [P, P], BF16)
    att1_p = dp("att1_t", [P, D], BF16)
    att2_p = dp("att2_t", [P, D], BF16)
    bias1_p = dp("bias1", [P, D])
    bias2_p = dp("bias2", [P, D])
    W2l_p = dp("W2l", [D, D], BF16)
    W2r_p = dp("W2r", [D, D], BF16)
    out_p = dp("out", [NPC, D], out=True)

    xl2_slab = nc.dram_tensor("xl2_slab", [NPC, D], BF16).ap()
    xl2_full = nc.dram_tensor("xl2_full", [pl.N, D], BF16,
                              addr_space="Shared").ap()
    xr2_loc = nc.dram_tensor("xr2_loc", [SLABP, D], BF16).ap()

    # per-block column offsets into the concatenated per-core arrays
    idx_off = np.cumsum([0] + [g * 8 for g in G])
    drt_off = np.cumsum([0] + list(G))
    drrow_off = np.cumsum([0] + [g * P for g in G])

    with TileContext(nc) as tc:
        nc.gpsimd.load_library(mlp)
        with (
            tc.tile_pool(name="const", bufs=1) as cpool,
            tc.tile_pool(name="stream", bufs=3) as spool,
            tc.tile_pool(name="work", bufs=2) as wpool,
            tc.tile_pool(name="small", bufs=3) as smpool,
            tc.tile_pool(name="psA", bufs=1, space="PSUM") as psA,
            tc.tile_pool(name="psB", bufs=2, space="PSUM") as psB,
            tc.tile_pool(name="psC", bufs=1, space="PSUM") as psC,
            tc.tile_pool(name="psD", bufs=1, space="PSUM") as psD,
        ):
            def cload(name, ap_in, shape, dt=F32):
                t = cpool.tile(shape, dt, tag=name)
                nc.sync.dma_start(out=t[:], in_=ap_in)
                return t

            iota_row_c = cload("c_iotar", iota_row_p[:, :], [P, P], BF16)
            iota_col_c = cload("c_iotac", iota_col_p[:, :], [P, 1])
            ones_c = cload("c_ones", ones_p[:, :], [1, P], BF16)
            ident_c = cload("c_ident", ident_p[:, :], [P, P], BF16)
            att1_c = cload("c_att1", att1_p[:, :], [P, D], BF16)
            att2_c = cload("c_att2", att2_p[:, :], [P, D], BF16)
            bias1_c = cload("c_bias1", bias1_p[:, :], [P, D])
            bias2_c = cload("c_bias2", bias2_p[:, :], [P, D])
            W2l_c = cload("c_w2l", W2l_p[:, :], [D, D], BF16)
            W2r_c = cload("c_w2r", W2r_p[:, :], [D, D], BF16)
            alpha_c = cpool.tile([P, 1], F32)
            nc.vector.memset(alpha_c[:], SLOPE)

            qctr = [0]

            def edge_layer(tab, xr_loc_ap, att_c, bias_c, layer1):
                for b in range(NBLK):
                    g, glo, ghi = G[b], G_lo[b], G_hi[b]
                    nreal = min(NPC - b * P, P)

                    idx_t = spool.tile([P, GMAX * 8], I16, tag="idx")
                    nc.sync.dma_start(
                        out=idx_t[:, 0:g * 8],
                        in_=idx_cat[:, idx_off[b]:idx_off[b] + g * 8])
                    drt_t = spool.tile([P, GMAX], BF16, tag="drt")
                    nc.sync.dma_start(
                        out=drt_t[:, 0:g],
                        in_=drt_cat[:, drt_off[b]:drt_off[b] + g])
                    xrb_t = smpool.tile([P, D], BF16, tag="xrb")
                    nc.sync.dma_start(out=xrb_t[:],
                                      in_=xr_loc_ap[b * P:(b + 1) * P, :])

                    # S[e, g, j] = (dst_rel[e] == j); ST[j, g, e] is its
                    # per-group transpose, built by one xbar DMA transpose.
                    S_t = wpool.tile([P, GMAX, P], BF16, tag="S", bufs=3)
                    nc.vector.tensor_tensor(
                        out=S_t[:, 0:g, :],
                        in0=iota_row_c[:].unsqueeze(1).to_broadcast([P, g, P]),
                        in1=drt_t[:, 0:g].unsqueeze(2).to_broadcast([P, g, P]),
                        op=mybir.AluOpType.is_equal)
                    ST_t = wpool.tile([P, GMAX, P], BF16, tag="ST")
                    nc.sync.dma_start_transpose(
                        out=ST_t[:, 0:g, :],
                        in_=S_t[:, 0:g, :].rearrange("p g j -> p (g j)"))

                    # A-side gather: xl[src] (bf16, 256B/edge); calls are
                    # chunked to stay within the per-queue descriptor ring.
                    za_t = wpool.tile([P, GMAX, D], BF16, tag="za", bufs=3)
                    for gn, g0, t0, t1 in ((glo, 0, 0, pl.SPLIT),
                                           (ghi, glo, pl.SPLIT, pl.N)):
                        for c0 in range(0, gn, gch):
                            cn = min(gch, gn - c0)
                            q0 = g0 + c0
                            nc.gpsimd.dma_gather(
                                out_ap=za_t[:, q0:q0 + cn, :],
                                in_ap=tab[t0:t1, :],
                                idxs_ap=idx_t[:, q0 * 8:(q0 + cn) * 8],
                                num_idxs=cn * P, num_idxs_reg=cn * P,
                                elem_size=D, queue_num=qctr[0] % queues)
                            qctr[0] += 1

                    # zb = ST_g @ xrb per group (PSUM) -> ACT copy to bf16;
                    # then ONE packed 2x DVE add z = za + zb.
                    zbS_t = wpool.tile([P, GMAX, D], BF16, tag="zbS")
                    for c8 in range(0, g, 8):
                        n8 = min(8, g - c8)
                        ps_zb = psB.tile([P, 8, D], F32, tag="pszb")
                        for i in range(n8):
                            gi = c8 + i
                            nc.tensor.matmul(
                                out=ps_zb[:, i, :],
                                lhsT=ST_t[:, gi, :],
                                rhs=xrb_t[:], start=True, stop=True)
                        nc.scalar.copy(out=zbS_t[:, c8:c8 + n8, :],
                                       in_=ps_zb[:, 0:n8, :])
                    z_t = zbS_t   # z = za + zb computed in place on zbS
                    nc.vector.tensor_tensor(
                        out=z_t[:, 0:g, :], in0=za_t[:, 0:g, :],
                        in1=zbS_t[:, 0:g, :], op=mybir.AluOpType.add)

                    # LeakyReLU -> att-mult -> C-sum tree, processed in two
                    # group-halves so ACT (prelu) and DVE (mult/tree) overlap.
                    e_t = smpool.tile([P, GMAX, H], BF16, tag="e")
                    halves = [(0, g // 2), (g // 2, g)] if g > 1 else [(0, g)]
                    for h0, h1 in halves:
                        hw_ = h1 - h0
                        if not hw_:
                            continue
                        nc.scalar.activation(
                            out=z_t[:, h0:h1, :], in_=z_t[:, h0:h1, :],
                            func=mybir.ActivationFunctionType.Prelu,
                            alpha=alpha_c[:, :])
                        nc.vector.tensor_tensor(
                            out=z_t[:, h0:h1, :], in0=z_t[:, h0:h1, :],
                            in1=att_c[:].unsqueeze(1).to_broadcast(
                                [P, hw_, D]),
                            op=mybir.AluOpType.mult)
                        zv = z_t[:, h0:h1, :].rearrange(
                            "p g (h c) -> p g h c", h=H)
                        with nc.allow_low_precision(
                                reason="bf16 e-scores; abs tol 2e-2 ok"):
                            for w in (16, 8, 4, 2):
                                nc.vector.tensor_tensor(
                                    out=zv[:, :, :, 0:w], in0=zv[:, :, :, 0:w],
                                    in1=zv[:, :, :, w:2 * w],
                                    op=mybir.AluOpType.add)
                            nc.vector.tensor_tensor(
                                out=e_t[:, h0:h1, :].unsqueeze(3),
                                in0=zv[:, :, :, 0:1],
                                in1=zv[:, :, :, 1:2], op=mybir.AluOpType.add)

                    # comb = [u | w]: w = exp(e) into cols D: (small) and
                    # expanded across C via a broadcast-input ACT exp (wex),
                    # so u = wex*za stays a packed 2x DVE mult.
                    comb_t = wpool.tile([P, GMAX, D + H], BF16, tag="comb")
                    nc.scalar.activation(out=comb_t[:, 0:g, D:D + H],
                                         in_=e_t[:, 0:g, :],
                                         func=mybir.ActivationFunctionType.Exp)
                    wex_t = wpool.tile([P, GMAX, D], BF16, tag="wex")
                    nc.scalar.activation(
                        out=wex_t[:, 0:g, :].rearrange(
                            "p g (h c) -> p g h c", h=H),
                        in_=e_t[:, 0:g, :].unsqueeze(3).to_broadcast(
                            [P, g, H, C]),
                        func=mybir.ActivationFunctionType.Exp)
                    nc.vector.tensor_tensor(
                        out=comb_t[:, 0:g, 0:D],
                        in0=za_t[:, 0:g, :],
                        in1=wex_t[:, 0:g, :],
                        op=mybir.AluOpType.mult)

                    # scatter: psum[j, D+H] += S_g^T @ comb_g
                    ps_agg = psC.tile([P, D + H], F32, tag="agg")
                    for gi in range(g):
                        nc.tensor.matmul(out=ps_agg[:],
                                         lhsT=S_t[:, gi, :],
                                         rhs=comb_t[:, gi, :],
                                         start=(gi == 0), stop=(gi == g - 1))

                    # epilogue
                    deps = smpool.tile([P, H], F32, tag="deps")
                    nc.vector.tensor_scalar_add(out=deps[:],
                                                in0=ps_agg[:, D:D + H],
                                                scalar1=1e-16)
                    dinv = smpool.tile([P, H], F32, tag="dinv")
                    nc.vector.reciprocal(out=dinv[:], in_=deps[:])
                    res = smpool.tile([P, D], F32, tag="res")
                    nc.vector.tensor_tensor(
                        out=res[:].rearrange("p (h c) -> p h c", h=H),
                        in0=ps_agg[:, 0:D].rearrange("p (h c) -> p h c", h=H),
                        in1=dinv[:].unsqueeze(2).to_broadcast([P, H, C]),
                        op=mybir.AluOpType.mult)
                    nc.vector.tensor_tensor(out=res[:], in0=res[:],
                                            in1=bias_c[:],
                                            op=mybir.AluOpType.add)

                    if layer1:
                        h_t = smpool.tile([P, D], BF16, tag="h")
                        nc.scalar.activation(
                            out=h_t[:], in_=res[:],
                            func=mybir.ActivationFunctionType.Relu)
                        psT = psD.tile([P, P], BF16, tag="xfm")
                        nc.tensor.transpose(out=psT[:], in_=h_t[:],
                                            identity=ident_c[:])
                        hT = smpool.tile([P, P], BF16, tag="hT")
                        nc.scalar.copy(out=hT[:], in_=psT[:])
                        for W_c, table, rows in ((W2l_c, xl2_slab, nreal),
                                                 (W2r_c, xr2_loc, P)):
                            psm = psD.tile([P, D], F32, tag="xfm2")
                            nc.tensor.matmul(out=psm[:], lhsT=hT[:],
                                             rhs=W_c[:], start=True, stop=True)
                            rt = smpool.tile([P, D], BF16, tag="xfm_o")
                            nc.scalar.copy(out=rt[:], in_=psm[:])
                            nc.sync.dma_start(
                                out=table[b * P:b * P + rows, :],
                                in_=rt[0:rows, :])
                    else:
                        h32 = smpool.tile([P, D], F32, tag="h32")
                        nc.scalar.activation(
                            out=h32[:], in_=res[:],
                            func=mybir.ActivationFunctionType.Relu)
                        nc.sync.dma_start(
                            out=out_p[b * P:b * P + nreal, :],
                            in_=h32[0:nreal, :])

            for _rep in range(repeat):
                edge_layer(xl1, xr1_loc, att1_c, bias1_c, layer1=True)
                if not skip_collective:
                    nc.gpsimd.collective_compute(
                        "AllGather", mybir.AluOpType.bypass,
                        replica_groups=[list(range(pl.NC))],
                        ins=[xl2_slab[:, :].opt()],
                        outs=[xl2_full[:, :].opt()],
                    )
                edge_layer(xl2_full, xr2_loc, att2_c, bias2_c, layer1=False)

    return nc


def make_inputs(plan, datas, x, W1_l, W1_r, att1, b1, W2_l, W2_r, att2, b2):
    pl = plan
    GMAX = pl.GMAX
    x = np.asarray(x, np.float32)
    xl1 = (x @ np.asarray(W1_l, np.float32)).astype(NPBF)
    xr1 = (x @ np.asarray(W1_r, np.float32)).astype(NPBF)

    iota_col = np.arange(P, dtype=np.float32)[:, None]
    iota_row = np.tile(np.arange(P, dtype=np.float32)[None, :],
                       (P, 1)).astype(NPBF)
    ones_t = np.ones((1, P), NPBF)
    ident = np.eye(P, dtype=np.float32).astype(NPBF)
    att1_t = np.tile(np.asarray(att1, np.float32).reshape(1, D),
                     (P, 1)).astype(NPBF)
    att2_t = np.tile(np.asarray(att2, np.float32).reshape(1, D),
                     (P, 1)).astype(NPBF)
    bias1_t = np.tile(np.asarray(b1, np.float32).reshape(1, D), (P, 1))
    bias2_t = np.tile(np.asarray(b2, np.float32).reshape(1, D), (P, 1))

    SLABP = pl.NBLK * P
    in_maps = []
    for k in range(pl.NC):
        xr1_loc = np.zeros((SLABP, D), NPBF)
        xr1_loc[:pl.NPC] = xr1[k * pl.NPC:(k + 1) * pl.NPC]
        in_maps.append(dict(
            xl1=xl1, xr1_loc=xr1_loc,
            idx_cat=datas[k]["idx_cat"],
            drt_cat=datas[k]["drt_cat"],
            drrow_cat=datas[k]["drrow_cat"],
            iota_col=iota_col, iota_row=iota_row, ones_t=ones_t, ident=ident,
            att1_t=att1_t, att2_t=att2_t,
            bias1=bias1_t.astype(np.float32), bias2=bias2_t.astype(np.float32),
            W2l=np.asarray(W2_l, np.float32).astype(NPBF),
            W2r=np.asarray(W2_r, np.float32).astype(NPBF),
        ))
    return in_maps


def assemble_output(plan, results):
    out = np.zeros((plan.N, D), np.float32)
    for k in range(plan.NC):
        out[k * plan.NPC:(k + 1) * plan.NPC] = results[k]["out"]
    return out


def kernel(x, edge_index, W1_l, W1_r, att1, b1, W2_l, W2_r, att2, b2):
    x = np.ascontiguousarray(np.asarray(x, np.float32))
    plan, datas = preprocess(x, np.asarray(edge_index), NC=8)
    nc = build_kernel(plan)
    nc.compile()
    in_maps = make_inputs(plan, datas, x, W1_l, W1_r, att1, b1,
                          W2_l, W2_r, att2, b2)
    res = run_bass_kernel_spmd(nc, in_maps, core_ids=list(range(8)))
    return assemble_output(plan, res.results)


# revision 36
# speedup vs baseline: 9.6436x; 1.2194x over previous
"""GATv2 2-layer GNN message-passing kernel for Trainium2, 8-core SPMD (v2).

Contract: kernel(**inputs) takes FULL unsharded inputs and returns the FULL
[50000, 128] float32 output.

Strategy (edge/data parallel, dst-range sharded, descriptor-minimized):
- Host: append self-loops, sort edges by dst; each core owns 6250 dst nodes
  = 49 blocks of 128. Per block, edges are split by src-half (int16 gather
  index limit) and padded to 128-edge groups with per-block group counts.
- All tables and per-edge compute are bf16 (fp32 PSUM accumulation);
  absmax-rel error ~5e-3 vs fp32 (gate 2e-2).
- Only ONE dma_gather stream per edge (xl[src], 256B descriptors) spread
  over 4 SWDGE queues with a 64KB descriptor ring. xr[dst] is NOT gathered:
  dst is block-local, so xr rows are reconstructed on the PE as
  zb = ST_g @ xrb  (ST = transposed one-hot of dst_rel built on-chip).
- Per-edge pipeline: z = za+zb (DVE), LeakyReLU (ACT), att-dot (DVE
  mult+reduce), w = exp (ACT, written into comb), u = w*za (DVE, into comb),
  then one PE matmul chain scatters S^T @ [u|w] into the block PSUM.
  out = relu(psum_u / (psum_w+eps) + bias).
- Layer 2 reuses the SAME gather indices: the AllGather writes xl2 in global
  node order ([50000,128] bf16), and xr2 stays core-local.
"""
import sys
sys.path.insert(0, '/opt/trn_rl_repo')
import numpy as np
from dataclasses import dataclass, field

import concourse.bass as bass
import concourse.bacc as bacc
import concourse.mybir as mybir
from concourse.tile import TileContext
from concourse.library_config import mlp
from concourse.bass_utils import run_bass_kernel_spmd

P = 128
H, C = 4, 32
D = H * C          # 128
SLOPE = 0.2
F32 = mybir.dt.float32
BF16 = mybir.dt.bfloat16
I16 = mybir.dt.int16
NPBF = mybir.dt.np(BF16)


@dataclass
class Plan:
    N: int
    NC: int
    NPC: int            # nodes per core (6250)
    NBLK: int           # blocks per core (49)
    SPLIT: int          # lo/hi table split (25000)
    G_lo: list = field(default_factory=list)   # per-block lo group count
    G_hi: list = field(default_factory=list)   # per-block hi group count

    @property
    def G(self):
        return [a + b for a, b in zip(self.G_lo, self.G_hi)]

    @property
    def GMAX(self):
        return max(self.G)


def wrap_idx(flat):
    """[n] int -> dma_gather SBUF idx layout [128, n//16]."""
    n = flat.shape[0]
    assert n % 16 == 0
    w = flat.reshape(n // 16, 16).T
    return np.tile(w, (8, 1)).astype(np.int16)


def preprocess(x, edge_index, NC=8):
    N = x.shape[0]
    NPC = N // NC
    NBLK = (NPC + P - 1) // P
    SPLIT = N // 2

    loop = np.arange(N, dtype=np.int64)
    src = np.concatenate([np.asarray(edge_index[0]), loop]).astype(np.int64)
    dst = np.concatenate([np.asarray(edge_index[1]), loop]).astype(np.int64)
    order = np.argsort(dst, kind='stable')
    src = src[order].astype(np.int32)
    dst = dst[order].astype(np.int32)
    core_bounds = np.searchsorted(dst, np.arange(NC + 1) * NPC)

    per_core = []
    lo_counts = np.zeros((NC, NBLK), np.int64)
    hi_counts = np.zeros((NC, NBLK), np.int64)
    for k in range(NC):
        a, b = core_bounds[k], core_bounds[k + 1]
        s_k = src[a:b]
        d_k = dst[a:b] - k * NPC
        blk = d_k // P
        is_lo = s_k < SPLIT
        lo_counts[k] = np.bincount(blk[is_lo], minlength=NBLK)
        hi_counts[k] = np.bincount(blk[~is_lo], minlength=NBLK)
        per_core.append((s_k, d_k, blk, is_lo))

    plan = Plan(N=N, NC=NC, NPC=NPC, NBLK=NBLK, SPLIT=SPLIT)
    plan.G_lo = [int(v) for v in
                 (lo_counts.max(axis=0) + P - 1) // P]
    plan.G_hi = [int(v) for v in
                 (hi_counts.max(axis=0) + P - 1) // P]
    G = plan.G

    datas = []
    for k in range(NC):
        s_k, d_k, blk, is_lo = per_core[k]
        idx_cols, drt_cols, drrow_cols = [], [], []
        for b in range(NBLK):
            in_b = blk == b
            dr_flat = np.full(G[b] * P, -1.0, np.float32)
            wrapped = []
            for side, gn, off in ((True, plan.G_lo[b], 0),
                                  (False, plan.G_hi[b], plan.G_lo[b] * P)):
                flat = np.zeros(gn * P, np.int16)
                sel = in_b & (is_lo == side)
                ss = s_k[sel]
                n = ss.shape[0]
                assert n <= gn * P, (k, b, side, n, gn)
                flat[:n] = ss if side else ss - SPLIT
                dr_flat[off:off + n] = d_k[sel] - b * P
                if gn:
                    wrapped.append(wrap_idx(flat))
            idx_cols.append(np.concatenate(wrapped, axis=1))
            drt_cols.append(dr_flat.reshape(G[b], P).T)   # [128, G]
            drrow_cols.append(dr_flat.reshape(1, G[b] * P))
        datas.append(dict(
            idx_cat=np.ascontiguousarray(np.concatenate(idx_cols, axis=1)),
            drt_cat=np.ascontiguousarray(
                np.concatenate(drt_cols, axis=1)).astype(NPBF),
            drrow_cat=np.ascontiguousarray(
                np.concatenate(drrow_cols, axis=1)).astype(NPBF),
        ))
    return plan, datas


def build_kernel(plan, repeat=1, skip_collective=False, queues=4,
                 scratch=65536, gch=8):
    pl = plan
    NBLK, NPC, GMAX = pl.NBLK, pl.NPC, pl.GMAX
    G, G_lo, G_hi = pl.G, pl.G_lo, pl.G_hi
    SLABP = NBLK * P                      # padded slab rows (6272)
    IDXW = sum(g * 8 for g in G)
    DRW = sum(G)
    ROWW = sum(g * P for g in G)
    # the SWDGE ring appears hard-limited to 1024 idxs (64 descs/lane) per
    # call regardless of scratch size — keep calls at <=1024 idxs (gch<=8)
    assert gch * P <= 1024
    assert scratch // 16 // queues >= gch * P

    nc = bacc.Bacc("TRN2", target_bir_lowering=False, debug=False,
                   dynamic_dma_scratch_size=scratch, num_swdge_queues=queues)
    dp = lambda name, shape, dt=F32, out=False: nc.declare_dram_parameter(
        name, list(shape), dt, isOutput=out).ap()

    xl1 = dp("xl1", [pl.N, D], BF16)
    xr1_loc = dp("xr1_loc", [SLABP, D], BF16)
    idx_cat = dp("idx_cat", [P, IDXW], I16)
    drt_cat = dp("drt_cat", [P, DRW], BF16)
    drrow_cat = dp("drrow_cat", [1, ROWW], BF16)
    iota_row_p = dp("iota_row", [P, P], BF16)
    iota_col_p = dp("iota_col", [P, 1])
    ones_p = dp("ones_t", [1, P], BF16)
    ident_p = dp("ident", [P, P], BF16)
    att1_p = dp("att1_t", [P, D], BF16)
    att2_p = dp("att2_t", [P, D], BF16)
    bias1_p = dp("bias1", [P, D])
    bias2_p = dp("bias2", [P, D])
    W2l_p = dp("W2l", [D, D], BF16)
    W2r_p = dp("W2r", [D, D], BF16)
    out_p = dp("out", [NPC, D], out=True)

    xl2_slab = nc.dram_tensor("xl2_slab", [NPC, D], BF16).ap()
    xl2_full = nc.dram_tensor("xl2_full", [pl.N, D], BF16,
                              addr_space="Shared").ap()
    xr2_loc = nc.dram_tensor("xr2_loc", [SLABP, D], BF16).ap()

    # per-block column offsets into the concatenated per-core arrays
    idx_off = np.cumsum([0] + [g * 8 for g in G])
    drt_off = np.cumsum([0] + list(G))
    drrow_off = np.cumsum([0] + [g * P for g in G])

    with TileContext(nc) as tc:
        nc.gpsimd.load_library(mlp)
        with (
            tc.tile_pool(name="const", bufs=1) as cpool,
            tc.tile_pool(name="stream", bufs=3) as spool,
            tc.tile_pool(name="work", bufs=2) as wpool,
            tc.tile_pool(name="small", bufs=3) as smpool,
            tc.tile_pool(name="psA", bufs=1, space="PSUM") as psA,
            tc.tile_pool(name="psB", bufs=2, space="PSUM") as psB,
            tc.tile_pool(name="psC", bufs=1, space="PSUM") as psC,
            tc.tile_pool(name="psD", bufs=1, space="PSUM") as psD,
        ):
            def cload(name, ap_in, shape, dt=F32):
                t = cpool.tile(shape, dt, tag=name)
                nc.sync.dma_start(out=t[:], in_=ap_in)
                return t

            iota_row_c = cload("c_iotar", iota_row_p[:, :], [P, P], BF16)
            iota_col_c = cload("c_iotac", iota_col_p[:, :], [P, 1])
            ones_c = cload("c_ones", ones_p[:, :], [1, P], BF16)
            ident_c = cload("c_ident", ident_p[:, :], [P, P], BF16)
            att1_c = cload("c_att1", att1_p[:, :], [P, D], BF16)
            att2_c = cload("c_att2", att2_p[:, :], [P, D], BF16)
            bias1_c = cload("c_bias1", bias1_p[:, :], [P, D])
            bias2_c = cload("c_bias2", bias2_p[:, :], [P, D])
            W2l_c = cload("c_w2l", W2l_p[:, :], [D, D], BF16)
            W2r_c = cload("c_w2r", W2r_p[:, :], [D, D], BF16)
            alpha_c = cpool.tile([P, 1], F32)
            nc.vector.memset(alpha_c[:], SLOPE)

            qctr = [0]

            def edge_layer(tab, xr_loc_ap, att_c, bias_c, layer1):
                for b in range(NBLK):
                    g, glo, ghi = G[b], G_lo[b], G_hi[b]
                    nreal = min(NPC - b * P, P)

                    idx_t = spool.tile([P, GMAX * 8], I16, tag="idx")
                    nc.sync.dma_start(
                        out=idx_t[:, 0:g * 8],
                        in_=idx_cat[:, idx_off[b]:idx_off[b] + g * 8])
                    drt_t = spool.tile([P, GMAX], BF16, tag="drt")
                    nc.sync.dma_start(
                        out=drt_t[:, 0:g],
                        in_=drt_cat[:, drt_off[b]:drt_off[b] + g])
                    xrb_t = smpool.tile([P, D], BF16, tag="xrb")
                    nc.sync.dma_start(out=xrb_t[:],
                                      in_=xr_loc_ap[b * P:(b + 1) * P, :])

                    # S[e, g, j] = (dst_rel[e] == j); ST[j, g, e] is its
                    # per-group transpose, built by one xbar DMA transpose.
                    S_t = wpool.tile([P, GMAX, P], BF16, tag="S", bufs=3)
                    gp = g // 3        # small share of the S build on Pool
                    nc.vector.tensor_tensor(
                        out=S_t[:, gp:g, :],
                        in0=iota_row_c[:].unsqueeze(1).to_broadcast(
                            [P, g - gp, P]),
                        in1=drt_t[:, gp:g].unsqueeze(2).to_broadcast(
                            [P, g - gp, P]),
                        op=mybir.AluOpType.is_equal)
                    if gp:
                        nc.gpsimd.tensor_tensor(
                            out=S_t[:, 0:gp, :],
                            in0=iota_row_c[:].unsqueeze(1).to_broadcast(
                                [P, gp, P]),
                            in1=drt_t[:, 0:gp].unsqueeze(2).to_broadcast(
                                [P, gp, P]),
                            op=mybir.AluOpType.is_equal)
                    ST_t = wpool.tile([P, GMAX, P], BF16, tag="ST")
                    nc.sync.dma_start_transpose(
                        out=ST_t[:, 0:g, :],
                        in_=S_t[:, 0:g, :].rearrange("p g j -> p (g j)"))

                    # A-side gather: xl[src] (bf16, 256B/edge); calls are
                    # chunked to stay within the per-queue descriptor ring.
                    za_t = wpool.tile([P, GMAX, D], BF16, tag="za", bufs=4)
                    for gn, g0, t0, t1 in ((glo, 0, 0, pl.SPLIT),
                                           (ghi, glo, pl.SPLIT, pl.N)):
                        for c0 in range(0, gn, gch):
                            cn = min(gch, gn - c0)
                            q0 = g0 + c0
                            nc.gpsimd.dma_gather(
                                out_ap=za_t[:, q0:q0 + cn, :],
                                in_ap=tab[t0:t1, :],
                                idxs_ap=idx_t[:, q0 * 8:(q0 + cn) * 8],
                                num_idxs=cn * P, num_idxs_reg=cn * P,
                                elem_size=D, queue_num=qctr[0] % queues)
                            qctr[0] += 1

                    # zb = ST_g @ xrb per group (PSUM) -> ACT copy to bf16;
                    # then ONE packed 2x DVE add z = za + zb.
                    zbS_t = wpool.tile([P, GMAX, D], BF16, tag="zbS")
                    for c8 in range(0, g, 8):
                        n8 = min(8, g - c8)
                        ps_zb = psB.tile([P, 8, D], F32, tag="pszb")
                        for i in range(n8):
                            gi = c8 + i
                            nc.tensor.matmul(
                                out=ps_zb[:, i, :],
                                lhsT=ST_t[:, gi, :],
                                rhs=xrb_t[:], start=True, stop=True)
                        nc.scalar.copy(out=zbS_t[:, c8:c8 + n8, :],
                                       in_=ps_zb[:, 0:n8, :])
                    z_t = zbS_t   # z = za + zb computed in place on zbS
                    nc.vector.tensor_tensor(
                        out=z_t[:, 0:g, :], in0=za_t[:, 0:g, :],
                        in1=zbS_t[:, 0:g, :], op=mybir.AluOpType.add)

                    # LeakyReLU -> att-mult -> C-sum tree, processed in two
                    # group-halves so ACT (prelu) and DVE (mult/tree) overlap.
                    e_t = smpool.tile([P, GMAX, H], BF16, tag="e")
                    halves = [(0, g // 2), (g // 2, g)] if g > 1 else [(0, g)]
                    for h0, h1 in halves:
                        hw_ = h1 - h0
                        if not hw_:
                            continue
                        nc.scalar.activation(
                            out=z_t[:, h0:h1, :], in_=z_t[:, h0:h1, :],
                            func=mybir.ActivationFunctionType.Prelu,
                            alpha=alpha_c[:, :])
                        nc.vector.tensor_tensor(
                            out=z_t[:, h0:h1, :], in0=z_t[:, h0:h1, :],
                            in1=att_c[:].unsqueeze(1).to_broadcast(
                                [P, hw_, D]),
                            op=mybir.AluOpType.mult)
                        zv = z_t[:, h0:h1, :].rearrange(
                            "p g (h c) -> p g h c", h=H)
                        with nc.allow_low_precision(
                                reason="bf16 e-scores; abs tol 2e-2 ok"):
                            for w in (16, 8, 4, 2):
                                nc.vector.tensor_tensor(
                                    out=zv[:, :, :, 0:w], in0=zv[:, :, :, 0:w],
                                    in1=zv[:, :, :, w:2 * w],
                                    op=mybir.AluOpType.add)
                            nc.vector.tensor_tensor(
                                out=e_t[:, h0:h1, :].unsqueeze(3),
                                in0=zv[:, :, :, 0:1],
                                in1=zv[:, :, :, 1:2], op=mybir.AluOpType.add)

                    # comb = [u | w]: w = exp(e) into cols D: (small) and
                    # expanded across C via a broadcast-input ACT exp (wex),
                    # so u = wex*za stays a packed 2x DVE mult.
                    comb_t = wpool.tile([P, GMAX, D + H], BF16, tag="comb")
                    nc.scalar.activation(out=comb_t[:, 0:g, D:D + H],
                                         in_=e_t[:, 0:g, :],
                                         func=mybir.ActivationFunctionType.Exp)
                    wex_t = wpool.tile([P, GMAX, D], BF16, tag="wex")
                    nc.scalar.activation(
                        out=wex_t[:, 0:g, :].rearrange(
                            "p g (h c) -> p g h c", h=H),
                        in_=e_t[:, 0:g, :].unsqueeze(3).to_broadcast(
                            [P, g, H, C]),
                        func=mybir.ActivationFunctionType.Exp)
                    nc.vector.tensor_tensor(
                        out=comb_t[:, 0:g, 0:D],
                        in0=za_t[:, 0:g, :],
                        in1=wex_t[:, 0:g, :],
                        op=mybir.AluOpType.mult)

                    # scatter: psum[j, D+H] += S_g^T @ comb_g
                    ps_agg = psC.tile([P, D + H], F32, tag="agg")
                    for gi in range(g):
                        nc.tensor.matmul(out=ps_agg[:],
                                         lhsT=S_t[:, gi, :],
                                         rhs=comb_t[:, gi, :],
                                         start=(gi == 0), stop=(gi == g - 1))

                    # epilogue
                    deps = smpool.tile([P, H], F32, tag="deps")
                    nc.vector.tensor_scalar_add(out=deps[:],
                                                in0=ps_agg[:, D:D + H],
                                                scalar1=1e-16)
                    dinv = smpool.tile([P, H], F32, tag="dinv")
                    nc.vector.reciprocal(out=dinv[:], in_=deps[:])
                    res = smpool.tile([P, D], F32, tag="res")
                    nc.vector.tensor_tensor(
                        out=res[:].rearrange("p (h c) -> p h c", h=H),
                        in0=ps_agg[:, 0:D].rearrange("p (h c) -> p h c", h=H),
                        in1=dinv[:].unsqueeze(2).to_broadcast([P, H, C]),
                        op=mybir.AluOpType.mult)
                    nc.vector.tensor_tensor(out=res[:], in0=res[:],
                                            in1=bias_c[:],
                                            op=mybir.AluOpType.add)

                    if layer1:
                        h_t = smpool.tile([P, D], BF16, tag="h")
                        nc.scalar.activation(
                            out=h_t[:], in_=res[:],
                            func=mybir.ActivationFunctionType.Relu)
                        psT = psD.tile([P, P], BF16, tag="xfm")
                        nc.tensor.transpose(out=psT[:], in_=h_t[:],
                                            identity=ident_c[:])
                        hT = smpool.tile([P, P], BF16, tag="hT")
                        nc.scalar.copy(out=hT[:], in_=psT[:])
                        for W_c, table, rows in ((W2l_c, xl2_slab, nreal),
                                                 (W2r_c, xr2_loc, P)):
                            psm = psD.tile([P, D], F32, tag="xfm2")
                            nc.tensor.matmul(out=psm[:], lhsT=hT[:],
                                             rhs=W_c[:], start=True, stop=True)
                            rt = smpool.tile([P, D], BF16, tag="xfm_o")
                            nc.scalar.copy(out=rt[:], in_=psm[:])
                            nc.sync.dma_start(
                                out=table[b * P:b * P + rows, :],
                                in_=rt[0:rows, :])
                    else:
                        h32 = smpool.tile([P, D], F32, tag="h32")
                        nc.scalar.activation(
                            out=h32[:], in_=res[:],
                            func=mybir.ActivationFunctionType.Relu)
                        nc.sync.dma_start(
                            out=out_p[b * P:b * P + nreal, :],
                            in_=h32[0:nreal, :])

            for _rep in range(repeat):
                edge_layer(xl1, xr1_loc, att1_c, bias1_c, layer1=True)
                if not skip_collective:
                    nc.gpsimd.collective_compute(
                        "AllGather", mybir.AluOpType.bypass,
                        replica_groups=[list(range(pl.NC))],
                        ins=[xl2_slab[:, :].opt()],
                        outs=[xl2_full[:, :].opt()],
                    )
                edge_layer(xl2_full, xr2_loc, att2_c, bias2_c, layer1=False)

    return nc


def make_inputs(plan, datas, x, W1_l, W1_r, att1, b1, W2_l, W2_r, att2, b2):
    pl = plan
    GMAX = pl.GMAX
    x = np.asarray(x, np.float32)
    xl1 = (x @ np.asarray(W1_l, np.float32)).astype(NPBF)
    xr1 = (x @ np.asarray(W1_r, np.float32)).astype(NPBF)

    iota_col = np.arange(P, dtype=np.float32)[:, None]
    iota_row = np.tile(np.arange(P, dtype=np.float32)[None, :],
                       (P, 1)).astype(NPBF)
    ones_t = np.ones((1, P), NPBF)
    ident = np.eye(P, dtype=np.float32).astype(NPBF)
    att1_t = np.tile(np.asarray(att1, np.float32).reshape(1, D),
                     (P, 1)).astype(NPBF)
    att2_t = np.tile(np.asarray(att2, np.float32).reshape(1, D),
                     (P, 1)).astype(NPBF)
    bias1_t = np.tile(np.asarray(b1, np.float32).reshape(1, D), (P, 1))
    bias2_t = np.tile(np.asarray(b2, np.float32).reshape(1, D), (P, 1))

    SLABP = pl.NBLK * P
    in_maps = []
    for k in range(pl.NC):
        xr1_loc = np.zeros((SLABP, D), NPBF)
        xr1_loc[:pl.NPC] = xr1[k * pl.NPC:(k + 1) * pl.NPC]
        in_maps.append(dict(
            xl1=xl1, xr1_loc=xr1_loc,
            idx_cat=datas[k]["idx_cat"],
            drt_cat=datas[k]["drt_cat"],
            drrow_cat=datas[k]["drrow_cat"],
            iota_col=iota_col, iota_row=iota_row, ones_t=ones_t, ident=ident,
            att1_t=att1_t, att2_t=att2_t,
            bias1=bias1_t.astype(np.float32), bias2=bias2_t.astype(np.float32),
            W2l=np.asarray(W2_l, np.float32).astype(NPBF),
            W2r=np.asarray(W2_r, np.float32).astype(NPBF),
        ))
    return in_maps


def assemble_output(plan, results):
    out = np.zeros((plan.N, D), np.float32)
    for k in range(plan.NC):
        out[k * plan.NPC:(k + 1) * plan.NPC] = results[k]["out"]
    return out


def kernel(x, edge_index, W1_l, W1_r, att1, b1, W2_l, W2_r, att2, b2):
    x = np.ascontiguousarray(np.asarray(x, np.float32))
    plan, datas = preprocess(x, np.asarray(edge_index), NC=8)
    nc = build_kernel(plan)
    nc.compile()
    in_maps = make_inputs(plan, datas, x, W1_l, W1_r, att1, b1,
                          W2_l, W2_r, att2, b2)
    res = run_bass_kernel_spmd(nc, in_maps, core_ids=list(range(8)))
    return assemble_output(plan, res.results)
